# revision 20
# baseline (speedup 1.0000x reference)
"""Trainium2 Bass kernel for the GWFFN spiking-CNN block (nn_GWFFN).

Reference computation (multi-step LIF scan over T=4, eval-mode BN):
  up:   LIF -> 1x1 conv (128->512) -> BN
  conv: LIF -> grouped 3x3 conv (512->512, groups=8, pad=1) -> BN -> +h1
  down: LIF -> 1x1 conv (512->128) -> BN -> +x

Sharding: data-parallel over batch N=16 -> 8 cores x 2 samples. Weights are
replicated, no collectives; each core runs an identical program on its slice.

Per-core layout: channels on SBUF partitions (the 512-wide inner dim is 4
tiles of 128), free dim = (n_loc=2, h=32, w=32) = 2048 positions, one
time-step at a time. Scaling the LIF state by 2 (v_t = w_t/2, u' = v after
reset) turns the scan into
    w_t = u'_{t-1} + x_t ;  s_t = (w_t >= 2) ;  u'_t = w_t * m_t
with m_t = (w_t < 2). The matmuls consume m instead of s via s = 1 - m
(weights negated, W@1 folded into per-channel biases applied in the ACT
PSUM->SBUF evacuation; the grouped conv's zero-pad border stores m=1 so the
ones-field is exact at the edges too).

Engine budget (per time-step, per core): PE does the matmuls (grouped conv
as 9 shifted 1x1 matmuls packed 4-at-a-time into 64x64 array quadrants);
ACT does the 16 PSUM evacuations; DVE does the LIF adds/mults and the s2
thresholds; Pool does the flat m1/m3 thresholds. The +h1 residual is PSUM-
preloaded via identity/half-swap matmuls for tiles 0-1 and DVE-added for
tiles 2-3 (PE/DVE balance knob). x arrives twice from the host: bf16
pre-scaled by 2^t for LIF1, and fp32 with the down-projection's ones-bias
pre-added for the final residual (so no rank-2 bias matmuls). Dead state
updates at t=T-1 are skipped.
"""

import numpy as np
import ml_dtypes

import concourse.bacc as bacc
import concourse.mybir as mybir
import concourse.tile as tile
from concourse.bass_utils import run_bass_kernel_spmd

FP32 = mybir.dt.float32
BF16 = mybir.dt.bfloat16
ALU = mybir.AluOpType
ACTF = mybir.ActivationFunctionType
BF = ml_dtypes.bfloat16

T, NFULL, C, H, W = 4, 16, 128, 32, 32
INNER, GROUPS = 512, 8
NCORES = 8
NLOC = NFULL // NCORES  # 2
HW = H * W              # 1024
F = NLOC * HW           # 2048 free positions per time-step
CH = 512                # matmul free-dim chunk (one PSUM bank fp32)
HP, WP = H + 2, W + 4   # padded spatial (W padded by 2 each side: 4B align)
EPS = 1e-5

# knobs. Odd ct tiles live partition-half-swapped in the LIF3 domain (the
# anti-diagonal conv packing emits them swapped), and the jm preload matrix
# is what applies that swap to h1 — so odd tiles must stay on the PE
# preload path; only even (natural-layout) tiles may use the DVE h1-add.
PRELOAD_H1 = (False, True, False, True)  # per-ct: +h1 via PE PSUM preload
M1_POOL = False       # Pool tensor_scalar measured ~31us per [128,2048]
M3_POOL = False       # op (50x slower than DVE) -- keep thresholds on DVE

_CACHE = {}


def _mm(nc, out, lhsT, rhs, start, stop):
    nc.tensor.matmul(out, lhsT, rhs, start=start, stop=stop,
                     skip_group_check=True)


def _build_nc():
    nc = bacc.Bacc("TRN2", target_bir_lowering=False)

    xs_d = nc.dram_tensor("xs", [T, C, F], BF16, kind="ExternalInput")
    xb_d = nc.dram_tensor("xb", [T, C, F], FP32, kind="ExternalInput")
    wup_d = nc.dram_tensor("wupT", [C, INNER], BF16, kind="ExternalInput")
    wcv_d = nc.dram_tensor("wconvP", [128, 2, 9, 128], BF16, kind="ExternalInput")
    wdn_d = nc.dram_tensor("wdnT", [128, 4, 128], BF16, kind="ExternalInput")
    jm_d = nc.dram_tensor("jmat", [128, 8, 128], BF16, kind="ExternalInput")
    bia_d = nc.dram_tensor("bias", [128, 32], FP32, kind="ExternalInput")
    o_d = nc.dram_tensor("out", [T, NLOC, C, H, W], FP32, kind="ExternalOutput")

    with tile.TileContext(nc) as tc:
        with (
            tc.tile_pool(name="const", bufs=1) as cpool,
            tc.tile_pool(name="state", bufs=1) as spool,
            tc.tile_pool(name="work", bufs=2) as wpool,
            tc.tile_pool(name="psum", bufs=4, space="PSUM") as ppool,
        ):
            # ---- t=0 input loads first: they gate the first LIF ------------
            xs0_sb = wpool.tile([128, F], BF16, tag="xs", bufs=2,
                                name="xs_0")
            nc.sync.dma_start(out=xs0_sb[:], in_=xs_d[0])
            wup_sb = cpool.tile([C, INNER], BF16)
            nc.sync.dma_start(out=wup_sb[:], in_=wup_d[:])
            bia_sb = cpool.tile([128, 32], FP32)
            nc.sync.dma_start(out=bia_sb[:], in_=bia_d[:])
            xb0_sb = wpool.tile([128, F], FP32, tag="xb", bufs=2,
                                name="xb_0")
            nc.sync.dma_start(out=xb0_sb[:], in_=xb_d[0])
            wcv_sb = cpool.tile([128, 2, 9, 128], BF16)
            nc.sync.dma_start(out=wcv_sb[:], in_=wcv_d[:])
            jm_sb = cpool.tile([128, 8, 128], BF16)
            nc.sync.dma_start(out=jm_sb[:], in_=jm_d[:])
            wdn_sb = cpool.tile([128, 4, 128], BF16)
            nc.sync.dma_start(out=wdn_sb[:], in_=wdn_d[:])

            # ---- persistent LIF state + padded m2 buffers ------------------
            u1 = spool.tile([128, F], BF16)
            u2, u3 = [], []
            s2pp = [[], []]
            for i in range(4):
                # u tiles need no memset: at t=0 every consumer reads the
                # aliased h/x tiles instead, and the first u write is a
                # full-tile overwrite
                u2t = spool.tile([128, F], BF16, name=f"u2_{i}")
                u2.append(u2t)
                u3t = spool.tile([128, F], BF16, name=f"u3_{i}")
                u3.append(u3t)
                for par in range(2):
                    # double-buffered by t parity so m2(t) never waits on
                    # conv(t-1) tap reads; border 1.0 == "no spike".
                    # All on GpSimd: it is otherwise idle, and this keeps
                    # the DVE/ACT queues clear for the t=0 critical path
                    # (par0 tiles emitted first -- needed at t=0)
                    s2t = spool.tile([128, NLOC, HP, WP], BF16,
                                     name=f"s2p_{par}_{i}")
                    s2pp[par].append(s2t)
            for par in range(2):
                for i in range(4):
                    nc.gpsimd.memset(s2pp[par][i][:], 1.0)

            # down-stage of step t-1, emitted after conv(t) so the PE
            # stream never waits on the t-1 LIF3 chain
            pend = None

            def emit_down(t, m3, xb_sb):
                out_sb = wpool.tile([128, F], FP32, tag="osb", bufs=2,
                                    name=f"osb_{t}")
                ps_dn = [ppool.tile([128, 2 * CH], FP32, tag="ps",
                                    name=f"psdn_{t}_{p}") for p in range(2)]
                for kt in range(4):       # kt-outer: 4 MMs reuse one LDW
                    for p in range(2):
                        for hh in range(2):
                            c = 2 * p + hh
                            _mm(nc, ps_dn[p][:, hh * CH:(hh + 1) * CH],
                                wdn_sb[:, kt, :],
                                m3[kt][:, c * CH:(c + 1) * CH],
                                start=(kt == 0), stop=(kt == 3))
                for p in range(2):
                    # final residual: xb already carries x + bias_dn
                    nc.vector.tensor_tensor(
                        out=out_sb[:, p * HW:(p + 1) * HW],
                        in0=ps_dn[p][:],
                        in1=xb_sb[:, p * HW:(p + 1) * HW],
                        op=ALU.add)
                    nc.sync.dma_start(
                        out=o_d[t, p].rearrange("c h w -> c h w"),
                        in_=out_sb.rearrange("p (n h w) -> p n h w",
                                             n=NLOC, h=H)[:, p])

            for t in range(T):
                s2p = s2pp[t % 2]
                last = t == T - 1
                # ---- load xs_t (bf16, pre-scaled by 2^t) and xb_t --------
                if t == 0:
                    xs_sb, xb_sb = xs0_sb, xb0_sb
                else:
                    xs_sb = wpool.tile([128, F], BF16, tag="xs", bufs=2,
                                       name=f"xs_{t}")
                    nc.sync.dma_start(out=xs_sb[:], in_=xs_d[t])
                    xb_sb = wpool.tile([128, F], FP32, tag="xb", bufs=2,
                                       name=f"xb_{t}")
                    nc.sync.dma_start(out=xb_sb[:], in_=xb_d[t])

                # ---- LIF1 (bf16, 2^t-scaled) -----------------------------
                if t == 0:
                    w1 = xs_sb
                else:
                    w1 = wpool.tile([128, F], BF16, tag="w1", bufs=1,
                                    name=f"w1_{t}")
                    nc.vector.tensor_tensor(
                        out=w1[:], in0=u1[:], in1=xs_sb[:], op=ALU.add)
                m1 = wpool.tile([128, F], BF16, tag="m1", bufs=1,
                                name=f"m1_{t}")
                m1eng = nc.gpsimd if M1_POOL else nc.vector
                m1eng.tensor_scalar(
                    out=m1[:], in0=w1[:], scalar1=float(2 ** (t + 1)),
                    scalar2=None, op0=ALU.is_lt)
                if not last:
                    nc.vector.tensor_tensor(
                        out=u1[:], in0=w1[:], in1=m1[:], op=ALU.mult)

                # ---- interleaved up-pairs and conv-quads -----------------
                # PE order: up(0),up(1), conv(q0), up(2),up(3), conv(q1) so
                # the DVE w2->s2 chain of one pair overlaps PE work on the
                # other; the h1 PSUM preloads go AFTER the taps so conv(q)
                # never waits on the pair's own up-evacs
                h1 = [None] * 4
                h2 = [None] * 4
                tmp3 = [None] * 4
                m3 = [None] * 4

                def emit_up(ct):
                    h1t = wpool.tile([128, F], BF16, tag="hbuf", bufs=7,
                                     name=f"h1_{t}_{ct}")
                    h1[ct] = h1t
                    for p in range(2):
                        ps_up = ppool.tile([128, 2 * CH], FP32, tag="ps",
                                           name=f"psup_{t}_{ct}_{p}")
                        for hh in range(2):
                            c = 2 * p + hh
                            _mm(nc, ps_up[:, hh * CH:(hh + 1) * CH],
                                wup_sb[:, 128 * ct:128 * (ct + 1)],
                                m1[:, c * CH:(c + 1) * CH],
                                start=True, stop=True)
                        nc.scalar.activation(
                            out=h1t[:, p * HW:(p + 1) * HW], in_=ps_up[:],
                            func=ACTF.Identity,
                            bias=bia_sb[:, 8 * t + ct:8 * t + ct + 1],
                            scale=float(2 ** t))
                    if t == 0:
                        w2t = h1t
                    else:
                        w2t = wpool.tile([128, F], BF16, tag="wbuf", bufs=5,
                                         name=f"w2_{t}_{ct}")
                        nc.vector.tensor_tensor(
                            out=w2t[:], in0=u2[ct][:], in1=h1t[:], op=ALU.add)
                    nc.vector.tensor_scalar(
                        out=s2p[ct][:, :, 1:1 + H, 2:2 + W],
                        in0=w2t.rearrange("p (n h w) -> p n h w", n=NLOC, h=H),
                        scalar1=float(2 ** (t + 1)), scalar2=None,
                        op0=ALU.is_lt)
                    if not last:
                        nc.vector.tensor_tensor(
                            out=u2[ct].rearrange("p (n h w) -> p n h w",
                                                 n=NLOC, h=H),
                            in0=w2t.rearrange("p (n h w) -> p n h w",
                                              n=NLOC, h=H),
                            in1=s2p[ct][:, :, 1:1 + H, 2:2 + W],
                            op=ALU.mult)
                    # early part of LIF3's 3-way add for non-preloaded
                    # tiles: tmp = u3 + h1 -- ready long before conv-evac
                    if not PRELOAD_H1[ct]:
                        if t == 0:
                            tmp3[ct] = h1t
                        else:
                            tt = wpool.tile([128, F], BF16, tag="tmp3",
                                            bufs=2, name=f"tmp3_{t}_{ct}")
                            nc.vector.tensor_tensor(
                                out=tt[:], in0=u3[ct][:], in1=h1t[:],
                                op=ALU.add)
                            tmp3[ct] = tt

                conv_ps = {}

                def emit_conv_mm(q):
                    # matmuls only -- the evacs are emitted separately so
                    # the strict-FIFO ACT queue isn't blocked behind the
                    # 4us conv accumulation while up-evacs still pend
                    ta, tb = 2 * q, 2 * q + 1   # s2 tiles feeding this quad
                    pre_a, pre_b = PRELOAD_H1[ta], PRELOAD_H1[tb]
                    for p in range(2):
                        P1 = ppool.tile([128, 2 * CH], FP32, tag="ps",
                                        name=f"psc1_{t}_{q}_{p}")
                        P2 = ppool.tile([128, 2 * CH], FP32, tag="ps",
                                        name=f"psc2_{t}_{q}_{p}")
                        conv_ps[(q, p)] = (P1, P2)
                        if pre_a:
                            for hh in range(2):
                                c = 2 * p + hh
                                sl = slice(c * CH, (c + 1) * CH)
                                _mm(nc, P1[:, hh * CH:(hh + 1) * CH],
                                    jm_sb[:, 2 * t, :], h1[ta][:, sl],
                                    start=True, stop=False)
                        if pre_b:
                            for hh in range(2):
                                c = 2 * p + hh
                                sl = slice(c * CH, (c + 1) * CH)
                                _mm(nc, P2[:, hh * CH:(hh + 1) * CH],
                                    jm_sb[:, 2 * t + 1, :], h1[tb][:, sl],
                                    start=True, stop=False)
                        for tap in range(9):
                            dy, dx = tap // 3, tap % 3
                            first = tap == 0
                            lastt = tap == 8
                            sa = first and not pre_a
                            sb = first and not pre_b
                            wq = wcv_sb[:, q, tap, :]
                            for hh in range(2):
                                h0 = 16 * hh
                                osl = slice(hh * CH, (hh + 1) * CH)
                                ra = s2p[ta][:, p, h0 + dy:h0 + dy + 16,
                                             1 + dx:33 + dx]
                                rb = s2p[tb][:, p, h0 + dy:h0 + dy + 16,
                                             1 + dx:33 + dx]
                                # T1: group 4q   rows 0-63  -> P1[0:64]
                                _mm(nc, P1[0:64, osl], wq[0:64, 0:64],
                                    ra[0:64], start=sa, stop=lastt)
                                # T2: group 4q+1 rows 64-127 -> P1[64:128]
                                _mm(nc, P1[64:128, osl], wq[64:128, 64:128],
                                    ra[64:128], start=sa, stop=lastt)
                                # T3: group 4q+2 rows 0-63  -> P2[64:128]
                                _mm(nc, P2[64:128, osl], wq[0:64, 64:128],
                                    rb[0:64], start=sb, stop=lastt)
                                # T4: group 4q+3 rows 64-127 -> P2[0:64]
                                _mm(nc, P2[0:64, osl], wq[64:128, 0:64],
                                    rb[64:128], start=sb, stop=lastt)

                def emit_conv_evac(q):
                    ta, tb = 2 * q, 2 * q + 1
                    h2a = wpool.tile([128, F], BF16, tag="hbuf", bufs=7,
                                     name=f"h2_{t}_{ta}")
                    h2b = wpool.tile([128, F], BF16, tag="hbuf", bufs=7,
                                     name=f"h2_{t}_{tb}")
                    h2[ta], h2[tb] = h2a, h2b
                    for p in range(2):
                        P1, P2 = conv_ps[(q, p)]
                        nc.scalar.activation(
                            out=h2a[:, p * HW:(p + 1) * HW], in_=P1[:],
                            func=ACTF.Identity,
                            bias=bia_sb[:, 8 * t + 4 + ta:8 * t + 5 + ta],
                            scale=float(2 ** t))
                        nc.scalar.activation(
                            out=h2b[:, p * HW:(p + 1) * HW], in_=P2[:],
                            func=ACTF.Identity,
                            bias=bia_sb[:, 8 * t + 4 + tb:8 * t + 5 + tb],
                            scale=float(2 ** t))

                def emit_lif3(ct):
                    if PRELOAD_H1[ct]:
                        # h2 already contains +h1 (PSUM preload)
                        if t == 0:
                            w3t = h2[ct]
                        else:
                            w3t = wpool.tile([128, F], BF16, tag="wbuf",
                                             bufs=5, name=f"w3_{t}_{ct}")
                            nc.vector.tensor_tensor(
                                out=w3t[:], in0=u3[ct][:], in1=h2[ct][:],
                                op=ALU.add)
                    else:
                        # tmp3 = u3 + h1 (or h1 at t=0); w3 = tmp3 + h2
                        w3t = wpool.tile([128, F], BF16, tag="wbuf",
                                         bufs=5, name=f"w3_{t}_{ct}")
                        nc.vector.tensor_tensor(
                            out=w3t[:], in0=tmp3[ct][:], in1=h2[ct][:],
                            op=ALU.add)
                    m3t = wpool.tile([128, F], BF16, tag="m3", bufs=5,
                                     name=f"m3_{t}_{ct}")
                    nc.vector.tensor_scalar(
                        out=m3t[:], in0=w3t[:],
                        scalar1=float(2 ** (t + 1)), scalar2=None,
                        op0=ALU.is_lt)
                    m3[ct] = m3t
                    if not last:
                        nc.vector.tensor_tensor(
                            out=u3[ct][:], in0=w3t[:], in1=m3t[:],
                            op=ALU.mult)

                # ups before convs: the shared 4-buf PSUM rotation requires
                # up tiles to recycle through ACT-evac'd buffers in queue
                # order (interleaving conv between up pairs deadlocks)
                emit_up(0)
                emit_up(1)
                emit_up(2)
                emit_up(3)
                # down stage of t-1 AFTER the up matmuls: its kt2/kt3 need
                # m3[2,3](t-1), which land ~4us after conv-q1(t-1) ends --
                # the up(t) matmuls (gated only on m1) fill that window.
                # PSUM-wise the two dn tiles become allocations #9-10 of
                # the cycle, recycling up tiles that ACT evac'd long ago.
                if pend is not None:
                    emit_down(*pend)
                emit_conv_mm(0)
                emit_conv_evac(0)
                emit_conv_mm(1)
                emit_conv_evac(1)
                emit_lif3(0)
                emit_lif3(1)
                emit_lif3(2)
                emit_lif3(3)

                pend = (t, m3, xb_sb)

            emit_down(*pend)

    nc.compile()
    return nc


def _prep_weights(inputs):
    """Fold BN into weights, apply the s = 1-m encoding (negate and compute
    per-channel ones-biases), pack/permute for the on-chip layout."""
    f32 = np.float32
    sc_up = (inputs["g_up"] / np.sqrt(inputs["v_up"] + EPS)).astype(f32)
    sc_cv = (inputs["g_conv"] / np.sqrt(inputs["v_conv"] + EPS)).astype(f32)
    sc_dn = (inputs["g_down"] / np.sqrt(inputs["v_down"] + EPS)).astype(f32)
    shifts = []
    for nm, sc in (("up", sc_up), ("conv", sc_cv), ("down", sc_dn)):
        shifts.append(inputs[f"b_{nm}"] - inputs[f"m_{nm}"] * sc)
    if max(np.abs(s).max() for s in shifts) > 0:
        raise NotImplementedError("nonzero BN shift not supported")

    w_up = np.asarray(inputs["w_up"], f32)[:, :, 0, 0] * sc_up[:, None]
    wupT = np.ascontiguousarray((-1.0 * w_up).T).astype(BF)    # [128, 512]
    # exact negative sum of the *rounded* weights: the dense m=1
    # background then cancels exactly and only spike terms carry bf16 error
    bias_up = -1.0 * wupT.astype(np.float64).sum(axis=0)       # [512]

    w_cv = np.asarray(inputs["w_conv"], f32) * sc_cv[:, None, None, None]
    wcvP = np.zeros((128, 2, 9, 128), f32)
    for q in range(2):
        for tap in range(9):
            dy, dx = tap // 3, tap % 3

            def blk(g):
                # W_g[ci, co] = -w_conv_eff[64g + co, ci, dy, dx]
                return np.ascontiguousarray(
                    -1.0 * w_cv[64 * g:64 * (g + 1), :, dy, dx].T)
            wcvP[0:64, q, tap, 0:64] = blk(4 * q)
            wcvP[64:128, q, tap, 64:128] = blk(4 * q + 1)
            wcvP[0:64, q, tap, 64:128] = blk(4 * q + 2)
            wcvP[64:128, q, tap, 0:64] = blk(4 * q + 3)
    wcvP = wcvP.astype(BF)
    # conv biases directly per psum partition m (P1 diag / P2 anti-diag),
    # again as exact negative half-sums of the rounded packed weights
    w64 = wcvP.astype(np.float64)
    bias_cv = np.zeros((4, 128))
    for q in range(2):
        lo = w64[0:64, q].sum(axis=(0, 1))     # [128] sum over rows<64, taps
        hi = w64[64:128, q].sum(axis=(0, 1))   # [128] sum over rows>=64
        bias_cv[2 * q] = -1.0 * np.concatenate([lo[:64], hi[64:]])
        bias_cv[2 * q + 1] = -1.0 * np.concatenate([hi[:64], lo[64:]])

    w_dn = np.asarray(inputs["w_down"], f32)[:, :, 0, 0] * sc_dn[:, None]
    # s3/m3 tile layouts: kt even natural, kt odd half-swapped ([g3|g2]...)
    wdnT = np.zeros((128, 4, 128), f32)
    for kt in range(4):
        rows = np.arange(128) + 128 * kt
        if kt % 2 == 1:
            rows = np.concatenate([rows[64:], rows[:64]])
        wdnT[:, kt, :] = -1.0 * w_dn[:, rows].T
    wdnT = wdnT.astype(BF)
    bias_dn = -1.0 * wdnT.astype(np.float64).sum(axis=(0, 1))  # [128]

    # per-t preload matrices: 2^-t * identity / half-swap (the h1 tiles
    # hold 2^t-scaled values; the preload rescales them back)
    jm = np.zeros((128, 8, 128), f32)
    for t in range(4):
        sc = 2.0 ** -t
        jm[np.arange(128), 2 * t, np.arange(128)] = sc
        jm[np.arange(128), 2 * t + 1, (np.arange(128) + 64) % 128] = sc
    jm = jm.astype(BF)

    # bias tile [128, 32]: cols 8t+0..3 = up bias per tile; 8t+4..7 = conv
    # bias per conv-out tile (odd tiles half-swapped to match the P2 psum
    # layout); ACT computes 2^t*psum + bias so the bias carries 2^t too
    bias = np.zeros((128, 32), f32)
    for t in range(4):
        sc = 2.0 ** t
        for ct in range(4):
            bias[:, 8 * t + ct] = sc * bias_up[128 * ct:128 * (ct + 1)]
            bias[:, 8 * t + 4 + ct] = sc * bias_cv[ct]

    return wupT, wcvP, wdnT, jm, bias, bias_dn.astype(f32)


def run(inputs, trace=False):
    if "nc" not in _CACHE:
        _CACHE["nc"] = _build_nc()
    nc = _CACHE["nc"]

    wupT, wcvP, wdnT, jm, bias, bias_dn = _prep_weights(inputs)
    x = np.asarray(inputs["x"], np.float32)
    # [T, N, C, H, W] -> [T, C, N, H, W] once, then slice per core
    xt = np.ascontiguousarray(x.transpose(0, 2, 1, 3, 4))
    scale_t = (2.0 ** np.arange(T, dtype=np.float32))[:, None, None]
    in_maps = []
    for i in range(NCORES):
        xc = xt[:, :, NLOC * i:NLOC * (i + 1)].reshape(T, C, F)
        xs = (xc * scale_t).astype(BF)
        xb = xc + bias_dn[None, :, None]
        in_maps.append({
            "xs": np.ascontiguousarray(xs),
            "xb": np.ascontiguousarray(xb),
            "wupT": wupT, "wconvP": wcvP, "wdnT": wdnT, "jmat": jm,
            "bias": bias,
        })
    res = run_bass_kernel_spmd(nc, in_maps, core_ids=list(range(NCORES)),
                               trace=trace)
    out = np.concatenate([r["out"] for r in res.results], axis=1)
    return out, res


def kernel(**inputs):
    out, _ = run(inputs, trace=False)
    return out


# revision 27
# speedup vs baseline: 1.0701x; 1.0701x over previous
"""Trainium2 Bass kernel for the GWFFN spiking-CNN block (nn_GWFFN).

Reference computation (multi-step LIF scan over T=4, eval-mode BN):
  up:   LIF -> 1x1 conv (128->512) -> BN
  conv: LIF -> grouped 3x3 conv (512->512, groups=8, pad=1) -> BN -> +h1
  down: LIF -> 1x1 conv (512->128) -> BN -> +x

Sharding: data-parallel over batch N=16 -> 8 cores x 2 samples. Weights are
replicated, no collectives; each core runs an identical program on its slice.

Per-core layout: channels on SBUF partitions (the 512-wide inner dim is 4
tiles of 128), free dim = (n_loc=2, h=32, w=32) = 2048 positions, one
time-step at a time. Scaling the LIF state by 2 (v_t = w_t/2, u' = v after
reset) turns the scan into
    w_t = u'_{t-1} + x_t ;  s_t = (w_t >= 2) ;  u'_t = w_t * m_t
with m_t = (w_t < 2). The matmuls consume m instead of s via s = 1 - m
(weights negated, W@1 folded into per-channel biases applied in the ACT
PSUM->SBUF evacuation; the grouped conv's zero-pad border stores m=1 so the
ones-field is exact at the edges too).

Engine budget (per time-step, per core): PE does the matmuls (grouped conv
as 9 shifted 1x1 matmuls packed 4-at-a-time into 64x64 array quadrants);
ACT does the 16 PSUM evacuations; DVE does the LIF adds/mults and the s2
thresholds; Pool does the flat m1/m3 thresholds. The +h1 residual is PSUM-
preloaded via identity/half-swap matmuls for tiles 0-1 and DVE-added for
tiles 2-3 (PE/DVE balance knob). x arrives twice from the host: bf16
pre-scaled by 2^t for LIF1, and fp32 with the down-projection's ones-bias
pre-added for the final residual (so no rank-2 bias matmuls). Dead state
updates at t=T-1 are skipped.
"""

import numpy as np
import ml_dtypes

import concourse.bacc as bacc
import concourse.mybir as mybir
import concourse.tile as tile
from concourse.bass_utils import run_bass_kernel_spmd

FP32 = mybir.dt.float32
BF16 = mybir.dt.bfloat16
ALU = mybir.AluOpType
ACTF = mybir.ActivationFunctionType
BF = ml_dtypes.bfloat16

T, NFULL, C, H, W = 4, 16, 128, 32, 32
INNER, GROUPS = 512, 8
NCORES = 8
NLOC = NFULL // NCORES  # 2
HW = H * W              # 1024
F = NLOC * HW           # 2048 free positions per time-step
CH = 512                # matmul free-dim chunk (one PSUM bank fp32)
HP, WP = H + 2, W + 4   # padded spatial (W padded by 2 each side: 4B align)
EPS = 1e-5

# knobs. Odd ct tiles live partition-half-swapped in the LIF3 domain (the
# anti-diagonal conv packing emits them swapped), and the jm preload matrix
# is what applies that swap to h1 — so odd tiles must stay on the PE
# preload path; only even (natural-layout) tiles may use the DVE h1-add.
PRELOAD_H1 = (False, True, False, True)  # per-ct: +h1 via PE PSUM preload
M1_POOL = False       # Pool tensor_scalar measured ~31us per [128,2048]
M3_POOL = False       # op (50x slower than DVE) -- keep thresholds on DVE

_CACHE = {}


def _mm(nc, out, lhsT, rhs, start, stop):
    nc.tensor.matmul(out, lhsT, rhs, start=start, stop=stop,
                     skip_group_check=True)


def _build_nc():
    nc = bacc.Bacc("TRN2", target_bir_lowering=False)

    xs_d = nc.dram_tensor("xs", [T, C, F], BF16, kind="ExternalInput")
    xb_d = nc.dram_tensor("xb", [T, C, F], FP32, kind="ExternalInput")
    wup_d = nc.dram_tensor("wupT", [C, INNER], BF16, kind="ExternalInput")
    wcv_d = nc.dram_tensor("wconvP", [128, 2, 9, 128], BF16, kind="ExternalInput")
    wdn_d = nc.dram_tensor("wdnT", [128, 4, 128], BF16, kind="ExternalInput")
    jm_d = nc.dram_tensor("jmat", [128, 8, 128], BF16, kind="ExternalInput")
    bia_d = nc.dram_tensor("bias", [128, 32], FP32, kind="ExternalInput")
    o_d = nc.dram_tensor("out", [T, NLOC, C, H, W], FP32, kind="ExternalOutput")

    with tile.TileContext(nc) as tc:
        with (
            tc.tile_pool(name="const", bufs=1) as cpool,
            tc.tile_pool(name="state", bufs=1) as spool,
            tc.tile_pool(name="work", bufs=2) as wpool,
            tc.tile_pool(name="psum", bufs=4, space="PSUM") as ppool,
        ):
            # ---- t=0 input loads first: they gate the first LIF ------------
            xs0_sb = wpool.tile([128, F], BF16, tag="xs", bufs=2,
                                name="xs_0")
            nc.sync.dma_start(out=xs0_sb[:], in_=xs_d[0])
            wup_sb = cpool.tile([C, INNER], BF16)
            nc.sync.dma_start(out=wup_sb[:], in_=wup_d[:])
            bia_sb = cpool.tile([128, 32], FP32)
            nc.sync.dma_start(out=bia_sb[:], in_=bia_d[:])
            xb0_sb = wpool.tile([128, F], FP32, tag="xb", bufs=2,
                                name="xb_0")
            nc.sync.dma_start(out=xb0_sb[:], in_=xb_d[0])
            wcv_sb = cpool.tile([128, 2, 9, 128], BF16)
            nc.sync.dma_start(out=wcv_sb[:], in_=wcv_d[:])
            jm_sb = cpool.tile([128, 8, 128], BF16)
            nc.sync.dma_start(out=jm_sb[:], in_=jm_d[:])
            wdn_sb = cpool.tile([128, 4, 128], BF16)
            nc.sync.dma_start(out=wdn_sb[:], in_=wdn_d[:])

            # ---- persistent LIF state + padded m2 buffers ------------------
            u1 = spool.tile([128, F], BF16)
            u2, u3 = [], []
            s2pp = [[], []]
            for i in range(4):
                # u tiles need no memset: at t=0 every consumer reads the
                # aliased h/x tiles instead, and the first u write is a
                # full-tile overwrite
                u2t = spool.tile([128, F], BF16, name=f"u2_{i}")
                u2.append(u2t)
                u3t = spool.tile([128, F], BF16, name=f"u3_{i}")
                u3.append(u3t)
                for par in range(2):
                    # double-buffered by t parity so m2(t) never waits on
                    # conv(t-1) tap reads; border 1.0 == "no spike".
                    # All on GpSimd: it is otherwise idle, and this keeps
                    # the DVE/ACT queues clear for the t=0 critical path
                    # (par0 tiles emitted first -- needed at t=0)
                    s2t = spool.tile([128, NLOC, HP, WP], BF16,
                                     name=f"s2p_{par}_{i}")
                    s2pp[par].append(s2t)
            for par in range(2):
                for i in range(4):
                    nc.gpsimd.memset(s2pp[par][i][:], 1.0)

            # down-stage of step t-1, emitted after conv(t) so the PE
            # stream never waits on the t-1 LIF3 chain
            pend = None

            def emit_down(t, m3, xb_sb):
                out_sb = wpool.tile([128, F], FP32, tag="osb", bufs=2,
                                    name=f"osb_{t}")
                ps_dn = [ppool.tile([128, 2 * CH], FP32, tag="ps",
                                    name=f"psdn_{t}_{p}") for p in range(2)]
                for kt in range(4):       # kt-outer: 4 MMs reuse one LDW
                    for p in range(2):
                        for hh in range(2):
                            c = 2 * p + hh
                            _mm(nc, ps_dn[p][:, hh * CH:(hh + 1) * CH],
                                wdn_sb[:, kt, :],
                                m3[kt][:, c * CH:(c + 1) * CH],
                                start=(kt == 0), stop=(kt == 3))
                for p in range(2):
                    # final residual: xb already carries x + bias_dn
                    nc.vector.tensor_tensor(
                        out=out_sb[:, p * HW:(p + 1) * HW],
                        in0=ps_dn[p][:],
                        in1=xb_sb[:, p * HW:(p + 1) * HW],
                        op=ALU.add)
                    nc.sync.dma_start(
                        out=o_d[t, p].rearrange("c h w -> c h w"),
                        in_=out_sb.rearrange("p (n h w) -> p n h w",
                                             n=NLOC, h=H)[:, p])

            for t in range(T):
                s2p = s2pp[t % 2]
                last = t == T - 1
                # ---- load xs_t (bf16, pre-scaled by 2^t) and xb_t --------
                if t == 0:
                    xs_sb, xb_sb = xs0_sb, xb0_sb
                else:
                    xs_sb = wpool.tile([128, F], BF16, tag="xs", bufs=2,
                                       name=f"xs_{t}")
                    nc.sync.dma_start(out=xs_sb[:], in_=xs_d[t])
                    xb_sb = wpool.tile([128, F], FP32, tag="xb", bufs=2,
                                       name=f"xb_{t}")
                    nc.sync.dma_start(out=xb_sb[:], in_=xb_d[t])

                # ---- LIF1 (bf16, 2^t-scaled) -----------------------------
                if t == 0:
                    w1 = xs_sb
                else:
                    w1 = wpool.tile([128, F], BF16, tag="w1", bufs=1,
                                    name=f"w1_{t}")
                    nc.vector.tensor_tensor(
                        out=w1[:], in0=u1[:], in1=xs_sb[:], op=ALU.add)
                m1 = wpool.tile([128, F], BF16, tag="m1", bufs=1,
                                name=f"m1_{t}")
                nc.vector.tensor_scalar(
                    out=m1[:], in0=w1[:], scalar1=float(2 ** (t + 1)),
                    scalar2=None, op0=ALU.is_lt)
                # state mults are deferred off the latency-critical DVE
                # chain (consumers only need them next time-step)
                deferred = []
                if not last:
                    deferred.append((u1[:], w1[:], m1[:]))

                def flush_deferred():
                    for out_ap, in0_ap, in1_ap in deferred:
                        nc.vector.tensor_tensor(
                            out=out_ap, in0=in0_ap, in1=in1_ap, op=ALU.mult)
                    deferred.clear()

                # ---- down stage of step t-1 (after LIF1 so m1(t) is ready
                # before the PE reaches up(t)) -----------------------------
                if pend is not None:
                    emit_down(*pend)

                # ---- interleaved up-pairs and conv-quads -----------------
                # PE order: up(0),up(1), conv(q0), up(2),up(3), conv(q1) so
                # the DVE w2->s2 chain of one pair overlaps PE work on the
                # other; the h1 PSUM preloads go AFTER the taps so conv(q)
                # never waits on the pair's own up-evacs
                h1 = [None] * 4
                h2 = [None] * 4
                tmp3 = [None] * 4
                m3 = [None] * 4

                def emit_up(ct):
                    h1t = wpool.tile([128, F], BF16, tag="hbuf", bufs=7,
                                     name=f"h1_{t}_{ct}")
                    h1[ct] = h1t
                    for p in range(2):
                        ps_up = ppool.tile([128, 2 * CH], FP32, tag="ps",
                                           name=f"psup_{t}_{ct}_{p}")
                        for hh in range(2):
                            c = 2 * p + hh
                            _mm(nc, ps_up[:, hh * CH:(hh + 1) * CH],
                                wup_sb[:, 128 * ct:128 * (ct + 1)],
                                m1[:, c * CH:(c + 1) * CH],
                                start=True, stop=True)
                        nc.scalar.activation(
                            out=h1t[:, p * HW:(p + 1) * HW], in_=ps_up[:],
                            func=ACTF.Identity,
                            bias=bia_sb[:, 8 * t + ct:8 * t + ct + 1],
                            scale=float(2 ** t))
                    if t == 0:
                        w2t = h1t
                    else:
                        w2t = wpool.tile([128, F], BF16, tag="wbuf", bufs=5,
                                         name=f"w2_{t}_{ct}")
                        nc.vector.tensor_tensor(
                            out=w2t[:], in0=u2[ct][:], in1=h1t[:], op=ALU.add)
                    nc.vector.tensor_scalar(
                        out=s2p[ct][:, :, 1:1 + H, 2:2 + W],
                        in0=w2t.rearrange("p (n h w) -> p n h w", n=NLOC, h=H),
                        scalar1=float(2 ** (t + 1)), scalar2=None,
                        op0=ALU.is_lt)
                    if not last:
                        deferred.append((
                            u2[ct].rearrange("p (n h w) -> p n h w",
                                             n=NLOC, h=H),
                            w2t.rearrange("p (n h w) -> p n h w",
                                          n=NLOC, h=H),
                            s2p[ct][:, :, 1:1 + H, 2:2 + W]))
                    # early part of LIF3's 3-way add for non-preloaded
                    # tiles: tmp = u3 + h1 -- ready long before conv-evac
                    if not PRELOAD_H1[ct]:
                        if t == 0:
                            tmp3[ct] = h1t
                        else:
                            tt = wpool.tile([128, F], BF16, tag="tmp3",
                                            bufs=2, name=f"tmp3_{t}_{ct}")
                            nc.vector.tensor_tensor(
                                out=tt[:], in0=u3[ct][:], in1=h1t[:],
                                op=ALU.add)
                            tmp3[ct] = tt

                conv_ps = {}

                def emit_conv_mm(q):
                    # matmuls only -- the evacs are emitted separately so
                    # the strict-FIFO ACT queue isn't blocked behind the
                    # 4us conv accumulation while up-evacs still pend
                    ta, tb = 2 * q, 2 * q + 1   # s2 tiles feeding this quad
                    pre_a, pre_b = PRELOAD_H1[ta], PRELOAD_H1[tb]
                    for p in range(2):
                        P1 = ppool.tile([128, 2 * CH], FP32, tag="ps",
                                        name=f"psc1_{t}_{q}_{p}")
                        P2 = ppool.tile([128, 2 * CH], FP32, tag="ps",
                                        name=f"psc2_{t}_{q}_{p}")
                        conv_ps[(q, p)] = (P1, P2)
                        if pre_a:
                            for hh in range(2):
                                c = 2 * p + hh
                                sl = slice(c * CH, (c + 1) * CH)
                                _mm(nc, P1[:, hh * CH:(hh + 1) * CH],
                                    jm_sb[:, 2 * t, :], h1[ta][:, sl],
                                    start=True, stop=False)
                        if pre_b:
                            for hh in range(2):
                                c = 2 * p + hh
                                sl = slice(c * CH, (c + 1) * CH)
                                _mm(nc, P2[:, hh * CH:(hh + 1) * CH],
                                    jm_sb[:, 2 * t + 1, :], h1[tb][:, sl],
                                    start=True, stop=False)
                        for tap in range(9):
                            dy, dx = tap // 3, tap % 3
                            first = tap == 0
                            lastt = tap == 8
                            sa = first and not pre_a
                            sb = first and not pre_b
                            wq = wcv_sb[:, q, tap, :]
                            for hh in range(2):
                                h0 = 16 * hh
                                osl = slice(hh * CH, (hh + 1) * CH)
                                ra = s2p[ta][:, p, h0 + dy:h0 + dy + 16,
                                             1 + dx:33 + dx]
                                rb = s2p[tb][:, p, h0 + dy:h0 + dy + 16,
                                             1 + dx:33 + dx]
                                # T1: group 4q   rows 0-63  -> P1[0:64]
                                _mm(nc, P1[0:64, osl], wq[0:64, 0:64],
                                    ra[0:64], start=sa, stop=lastt)
                                # T2: group 4q+1 rows 64-127 -> P1[64:128]
                                _mm(nc, P1[64:128, osl], wq[64:128, 64:128],
                                    ra[64:128], start=sa, stop=lastt)
                                # T3: group 4q+2 rows 0-63  -> P2[64:128]
                                _mm(nc, P2[64:128, osl], wq[0:64, 64:128],
                                    rb[0:64], start=sb, stop=lastt)
                                # T4: group 4q+3 rows 64-127 -> P2[0:64]
                                _mm(nc, P2[0:64, osl], wq[64:128, 0:64],
                                    rb[64:128], start=sb, stop=lastt)

                def emit_conv_evac(q):
                    ta, tb = 2 * q, 2 * q + 1
                    h2a = wpool.tile([128, F], BF16, tag="hbuf", bufs=7,
                                     name=f"h2_{t}_{ta}")
                    h2b = wpool.tile([128, F], BF16, tag="hbuf", bufs=7,
                                     name=f"h2_{t}_{tb}")
                    h2[ta], h2[tb] = h2a, h2b
                    for p in range(2):
                        P1, P2 = conv_ps[(q, p)]
                        nc.scalar.activation(
                            out=h2a[:, p * HW:(p + 1) * HW], in_=P1[:],
                            func=ACTF.Identity,
                            bias=bia_sb[:, 8 * t + 4 + ta:8 * t + 5 + ta],
                            scale=float(2 ** t))
                        nc.scalar.activation(
                            out=h2b[:, p * HW:(p + 1) * HW], in_=P2[:],
                            func=ACTF.Identity,
                            bias=bia_sb[:, 8 * t + 4 + tb:8 * t + 5 + tb],
                            scale=float(2 ** t))

                def lif3_w3(ct):
                    if PRELOAD_H1[ct]:
                        # h2 already contains +h1 (PSUM preload)
                        if t == 0:
                            return h2[ct]
                        w3t = wpool.tile([128, F], BF16, tag="wbuf",
                                         bufs=5, name=f"w3_{t}_{ct}")
                        nc.vector.tensor_tensor(
                            out=w3t[:], in0=u3[ct][:], in1=h2[ct][:],
                            op=ALU.add)
                        return w3t
                    # tmp3 = u3 + h1 (or h1 at t=0); w3 = tmp3 + h2
                    w3t = wpool.tile([128, F], BF16, tag="wbuf",
                                     bufs=5, name=f"w3_{t}_{ct}")
                    nc.vector.tensor_tensor(
                        out=w3t[:], in0=tmp3[ct][:], in1=h2[ct][:],
                        op=ALU.add)
                    return w3t

                def emit_lif3_pair(cta, ctb):
                    # w3,w3,m3,m3 first (m3 feeds next step's down matmuls
                    # == the t-boundary critical chain), u3 state mults last
                    w3s = [lif3_w3(cta), lif3_w3(ctb)]
                    for ct, w3t in zip((cta, ctb), w3s):
                        m3t = wpool.tile([128, F], BF16, tag="m3", bufs=5,
                                         name=f"m3_{t}_{ct}")
                        nc.vector.tensor_scalar(
                            out=m3t[:], in0=w3t[:],
                            scalar1=float(2 ** (t + 1)), scalar2=None,
                            op0=ALU.is_lt)
                        m3[ct] = m3t
                    if not last:
                        for ct, w3t in zip((cta, ctb), w3s):
                            nc.vector.tensor_tensor(
                                out=u3[ct][:], in0=w3t[:], in1=m3[ct][:],
                                op=ALU.mult)

                # ups before convs: the shared 4-buf PSUM rotation requires
                # up tiles to recycle through ACT-evac'd buffers in queue
                # order (interleaving conv between up pairs deadlocks)
                emit_up(0)
                emit_up(1)
                emit_up(2)
                emit_up(3)
                flush_deferred()    # u1/u2 state mults, off the s2 chain
                emit_conv_mm(0)
                emit_conv_evac(0)
                emit_lif3_pair(0, 1)
                emit_conv_mm(1)
                emit_conv_evac(1)
                emit_lif3_pair(2, 3)

                pend = (t, m3, xb_sb)

            emit_down(*pend)

    nc.compile()
    return nc


def _prep_weights(inputs):
    """Fold BN into weights, apply the s = 1-m encoding (negate and compute
    per-channel ones-biases), pack/permute for the on-chip layout."""
    f32 = np.float32
    sc_up = (inputs["g_up"] / np.sqrt(inputs["v_up"] + EPS)).astype(f32)
    sc_cv = (inputs["g_conv"] / np.sqrt(inputs["v_conv"] + EPS)).astype(f32)
    sc_dn = (inputs["g_down"] / np.sqrt(inputs["v_down"] + EPS)).astype(f32)
    shifts = []
    for nm, sc in (("up", sc_up), ("conv", sc_cv), ("down", sc_dn)):
        shifts.append(inputs[f"b_{nm}"] - inputs[f"m_{nm}"] * sc)
    if max(np.abs(s).max() for s in shifts) > 0:
        raise NotImplementedError("nonzero BN shift not supported")

    w_up = np.asarray(inputs["w_up"], f32)[:, :, 0, 0] * sc_up[:, None]
    wupT = np.ascontiguousarray((-1.0 * w_up).T).astype(BF)    # [128, 512]
    # exact negative sum of the *rounded* weights: the dense m=1
    # background then cancels exactly and only spike terms carry bf16 error
    bias_up = -1.0 * wupT.astype(np.float64).sum(axis=0)       # [512]

    w_cv = np.asarray(inputs["w_conv"], f32) * sc_cv[:, None, None, None]
    wcvP = np.zeros((128, 2, 9, 128), f32)
    for q in range(2):
        for tap in range(9):
            dy, dx = tap // 3, tap % 3

            def blk(g):
                # W_g[ci, co] = -w_conv_eff[64g + co, ci, dy, dx]
                return np.ascontiguousarray(
                    -1.0 * w_cv[64 * g:64 * (g + 1), :, dy, dx].T)
            wcvP[0:64, q, tap, 0:64] = blk(4 * q)
            wcvP[64:128, q, tap, 64:128] = blk(4 * q + 1)
            wcvP[0:64, q, tap, 64:128] = blk(4 * q + 2)
            wcvP[64:128, q, tap, 0:64] = blk(4 * q + 3)
    wcvP = wcvP.astype(BF)
    # conv biases directly per psum partition m (P1 diag / P2 anti-diag),
    # again as exact negative half-sums of the rounded packed weights
    w64 = wcvP.astype(np.float64)
    bias_cv = np.zeros((4, 128))
    for q in range(2):
        lo = w64[0:64, q].sum(axis=(0, 1))     # [128] sum over rows<64, taps
        hi = w64[64:128, q].sum(axis=(0, 1))   # [128] sum over rows>=64
        bias_cv[2 * q] = -1.0 * np.concatenate([lo[:64], hi[64:]])
        bias_cv[2 * q + 1] = -1.0 * np.concatenate([hi[:64], lo[64:]])

    w_dn = np.asarray(inputs["w_down"], f32)[:, :, 0, 0] * sc_dn[:, None]
    # s3/m3 tile layouts: kt even natural, kt odd half-swapped ([g3|g2]...)
    wdnT = np.zeros((128, 4, 128), f32)
    for kt in range(4):
        rows = np.arange(128) + 128 * kt
        if kt % 2 == 1:
            rows = np.concatenate([rows[64:], rows[:64]])
        wdnT[:, kt, :] = -1.0 * w_dn[:, rows].T
    wdnT = wdnT.astype(BF)
    bias_dn = -1.0 * wdnT.astype(np.float64).sum(axis=(0, 1))  # [128]

    # per-t preload matrices: 2^-t * identity / half-swap (the h1 tiles
    # hold 2^t-scaled values; the preload rescales them back)
    jm = np.zeros((128, 8, 128), f32)
    for t in range(4):
        sc = 2.0 ** -t
        jm[np.arange(128), 2 * t, np.arange(128)] = sc
        jm[np.arange(128), 2 * t + 1, (np.arange(128) + 64) % 128] = sc
    jm = jm.astype(BF)

    # bias tile [128, 32]: cols 8t+0..3 = up bias per tile; 8t+4..7 = conv
    # bias per conv-out tile (odd tiles half-swapped to match the P2 psum
    # layout); ACT computes 2^t*psum + bias so the bias carries 2^t too
    bias = np.zeros((128, 32), f32)
    for t in range(4):
        sc = 2.0 ** t
        for ct in range(4):
            bias[:, 8 * t + ct] = sc * bias_up[128 * ct:128 * (ct + 1)]
            bias[:, 8 * t + 4 + ct] = sc * bias_cv[ct]

    return wupT, wcvP, wdnT, jm, bias, bias_dn.astype(f32)


def run(inputs, trace=False):
    if "nc" not in _CACHE:
        _CACHE["nc"] = _build_nc()
    nc = _CACHE["nc"]

    wupT, wcvP, wdnT, jm, bias, bias_dn = _prep_weights(inputs)
    x = np.asarray(inputs["x"], np.float32)
    # [T, N, C, H, W] -> [T, C, N, H, W] once, then slice per core
    xt = np.ascontiguousarray(x.transpose(0, 2, 1, 3, 4))
    scale_t = (2.0 ** np.arange(T, dtype=np.float32))[:, None, None]
    in_maps = []
    for i in range(NCORES):
        xc = xt[:, :, NLOC * i:NLOC * (i + 1)].reshape(T, C, F)
        xs = (xc * scale_t).astype(BF)
        xb = xc + bias_dn[None, :, None]
        in_maps.append({
            "xs": np.ascontiguousarray(xs),
            "xb": np.ascontiguousarray(xb),
            "wupT": wupT, "wconvP": wcvP, "wdnT": wdnT, "jmat": jm,
            "bias": bias,
        })
    res = run_bass_kernel_spmd(nc, in_maps, core_ids=list(range(NCORES)),
                               trace=trace)
    out = np.concatenate([r["out"] for r in res.results], axis=1)
    return out, res


def kernel(**inputs):
    out, _ = run(inputs, trace=False)
    return out
